# revision 19
# baseline (speedup 1.0000x reference)
"""Trainium2 Bass kernel for nn_DHGNNLayer (gnn_message_passing).

Math (from the reference):
    h   = relu((B1 @ x) @ W1)          # [n_nodes, 128], B1 = COO incidence
    out = mean_e sigmoid((hw0[r_{2e}] + hw0[r_{2e+1}]) / 2)   # scalar
    where hw0 = h @ W2[:, 0]           # only column 0 is ever needed

Key facts used:
  - inc_cols == arange(NNZ)//2 -> every edge has exactly 2 nonzeros, deg == 2.
  - segment_sum commutes with the right-multiply: B1 @ (x@W1) = (B1@x) @ W1,
    so the scatter runs on raw x rows and W1 is applied per *node* (6.3k rows
    per core) instead of per nonzero (50k).

Strategy (8 cores, node-partition parallelism, no collectives):
  Launch A: host sorts nonzeros by destination node and cuts the sorted
    stream into blocks of <=16 consecutive nodes / <=128 nonzeros.  Each
    block is ONE self-contained matmul: stationary xg [128nnz, 128ch] fp8
    (gathered x rows), moving one-hot G [128nnz, 16] fp8 -> psum hx^T
    [128ch, 16 nodes].  32 blocks pack one psum bank [128, 512]; per strip:
    DVE copies psum->SBUF bf16, W1 matmul, scalar relu, w2c matvec.
    Small moving operands put each matmul near the ~40ns dispatch floor
    (vs ~134 cycles for 128-wide G), and G shrinks 8x.
  Launch B: host gathers hw0[inc_rows] (free), device does
    sigmoid(0.5*(a+b)) and reduces; host combines 8 partial sums.
"""

import numpy as np
import ml_dtypes

N_NODES = 50000
N_EDGES = 200000
C = 128
NNZ = 2 * N_EDGES
NCORES = 8
NBW = 16                       # node slots per block (psum col window)
TPS = 32                       # blocks (tiles) per psum strip [128, 512]
FP8_ONE = np.uint8(0x38)       # float8_e4m3 encoding of 1.0

_PROGS = {}
TRACE = False
LAST = {}


def _bacc():
    import concourse.bacc as bacc

    return bacc.Bacc("TRN2", target_bir_lowering=False, debug=False,
                     num_devices=NCORES)


def _build_prog_a(nt):
    """Layer-1 program: per block one matmul; per strip W1 + relu + w2c."""
    import concourse.mybir as mybir
    from concourse import tile

    dtb = mybir.dt.bfloat16
    dtf = mybir.dt.float32
    dt8 = mybir.dt.float8e4
    AF = mybir.ActivationFunctionType
    nstrip = nt // TPS

    nc = _bacc()
    xg_d = nc.dram_tensor("xg", [128, nt, C + NBW], dt8,
                          kind="ExternalInput")
    w1_d = nc.dram_tensor("w1", [C, C], dtb, kind="ExternalInput")
    w2c_d = nc.dram_tensor("w2c", [C, 32], dtb, kind="ExternalInput")
    hw0_d = nc.dram_tensor("hw0", [1, nt * NBW], dtf, kind="ExternalOutput")

    with tile.TileContext(nc) as tc:
        with (
            tc.tile_pool(name="const", bufs=1) as constp,
            tc.tile_pool(name="xgp", bufs=7) as xgp,
            tc.tile_pool(name="hxp", bufs=3) as hxp,
            tc.tile_pool(name="rlp", bufs=3) as rlp,
            tc.tile_pool(name="ps_hx", bufs=4, space="PSUM") as ps_hx,
            tc.tile_pool(name="ps_h", bufs=2, space="PSUM") as ps_h,
            tc.tile_pool(name="ps_o", bufs=2, space="PSUM") as ps_o,
        ):
            # first strip's xg||g, then the small constants (FIFO)
            xg_t = xgp.tile([128, TPS, C + NBW], dt8, tag="xg")
            nc.sync.dma_start(xg_t[:], xg_d[:, 0:TPS, :])
            w1_sb = constp.tile([C, C], dtb)
            nc.sync.dma_start(w1_sb[:], w1_d[:])
            w2c_sb = constp.tile([C, 32], dtb)
            nc.sync.dma_start(w2c_sb[:], w2c_d[:])
            hw0_sb = constp.tile([1, nt * NBW], dtf)

            # software-pipelined: tail matmuls for strip s issue 1-2 strips
            # late so the tensor engine never waits on DVE/scalar
            hxT_q = {}
            relu_q = {}
            for s in range(nstrip + 2):
                if s < nstrip:
                    if s > 0:
                        xg_t = xgp.tile([128, TPS, C + NBW], dt8, tag="xg")
                        nc.sync.dma_start(xg_t[:],
                                          xg_d[:, s * TPS:(s + 1) * TPS, :])
                    ps = ps_hx.tile([128, TPS * NBW], dtf, tag="hx")
                    for i in range(TPS):
                        nc.tensor.matmul(ps[:, i * NBW:(i + 1) * NBW],
                                         xg_t[:, i, :C],
                                         xg_t[:, i, C:C + NBW],
                                         start=True, stop=True)
                    hxT = hxp.tile([128, TPS * NBW], dtb, tag="hxT")
                    nc.vector.tensor_copy(hxT[:], ps[:])
                    hxT_q[s] = hxT
                if s >= 1 and s - 1 < nstrip:
                    psh = ps_h.tile([128, TPS * NBW], dtf, tag="h")
                    nc.tensor.matmul(psh[:], w1_sb[:], hxT_q.pop(s - 1)[:],
                                     start=True, stop=True)
                    reluT = rlp.tile([128, TPS * NBW], dtb, tag="reluT")
                    nc.scalar.activation(reluT[:], psh[:], AF.Relu)
                    relu_q[s - 1] = reluT
                if s >= 2 and s - 2 < nstrip:
                    # M padded to 32: a [1,512] psum write is an extreme
                    # thin-M matmul and stalls the PE write path
                    pso = ps_o.tile([32, TPS * NBW], dtf, tag="o")
                    nc.tensor.matmul(pso[:], w2c_sb[:],
                                     relu_q.pop(s - 2)[:],
                                     start=True, stop=True)
                    nc.scalar.activation(
                        hw0_sb[:, (s - 2) * TPS * NBW:(s - 1) * TPS * NBW],
                        pso[0:1, :], AF.Copy)

            nc.sync.dma_start(hw0_d[:], hw0_sb[:])

    nc.compile()
    return nc


def _build_prog_b(free):
    """Layer-2 program (raw bass, minimal tail):
    acc[p] = sum_f sigmoid(0.5*(a+b)).  zab is [za | zb] along free."""
    import concourse.bass as bass
    import concourse.mybir as mybir

    dtb = mybir.dt.bfloat16
    dtf = mybir.dt.float32
    AF = mybir.ActivationFunctionType

    nc = bass.Bass()
    zab_d = nc.dram_tensor("zab", [128, 2 * free], dtb, kind="ExternalInput")
    acc_d = nc.dram_tensor("acc", [128, 1], dtf, kind="ExternalOutput")

    with (
        nc.sbuf_tensor([128, 2 * free], dtb) as zab_sb,
        nc.sbuf_tensor([128, free], dtf) as t_sb,
        nc.sbuf_tensor([128, free], dtf) as s_sb,
        nc.sbuf_tensor([128, 1], dtf) as r_sb,
        nc.semaphore() as dsem,
        nc.semaphore() as csem,
        nc.Block() as block,
    ):
        @block.sync
        def _(sync):
            sync.dma_start(zab_sb[:], zab_d[:]).then_inc(dsem, 16)
            sync.wait_ge(csem, 2)
            sync.dma_start(acc_d[:], r_sb[:]).then_inc(dsem, 16)

        @block.vector
        def _(vector):
            vector.wait_ge(dsem, 16)
            nc.vector.tensor_add(t_sb[:], zab_sb[:, :free],
                                 zab_sb[:, free:]).then_inc(csem, 1)

        @block.scalar
        def _(scalar):
            # dummy op preloads the sigmoid table during the input DMA
            nc.scalar.activation(s_sb[:, 0:1], t_sb[:, 0:1], AF.Sigmoid)
            scalar.wait_ge(csem, 1)
            nc.scalar.activation(s_sb[:], t_sb[:], AF.Sigmoid, scale=0.5,
                                 accum_out=r_sb[:]).then_inc(csem, 1)

    return nc


def _get_prog(key, builder, *args):
    if key not in _PROGS:
        _PROGS[key] = builder(*args)
    return _PROGS[key]


def _run(nc, in_maps, tag):
    from concourse.bass_utils import run_bass_kernel_spmd
    import time

    t0 = time.perf_counter()
    res = run_bass_kernel_spmd(nc, in_maps, list(range(NCORES)), trace=TRACE)
    LAST[tag + "_wall_s"] = time.perf_counter() - t0
    LAST[tag + "_exec_ns"] = res.exec_time_ns
    return res.results


def kernel(x, w1, w2, inc_rows, inc_cols, n_nodes=None, n_edges=None):
    x = np.asarray(x, dtype=np.float32)
    w1 = np.asarray(w1, dtype=np.float32)
    w2 = np.asarray(w2, dtype=np.float32)
    inc_rows = np.asarray(inc_rows)
    inc_cols = np.asarray(inc_cols)
    assert x.shape == (N_EDGES, C) and inc_rows.shape == (NNZ,)
    assert np.array_equal(inc_cols.astype(np.int64),
                          np.arange(NNZ, dtype=np.int64) // 2)

    # ---- host prep: sort nnz by destination node, cut variable blocks ----
    order = np.argsort(inc_rows, kind="stable")
    rs = inc_rows[order].astype(np.int64)
    cs = (order >> 1).astype(np.int64)          # edge id per nonzero

    counts = np.bincount(rs, minlength=N_NODES)
    nz_nodes = np.flatnonzero(counts)
    cnt = counts[nz_nodes]
    assert cnt.max() <= 128

    # greedy cut: <=16 nodes and <=128 nnz per block
    blk_first = []      # first nz-node rank in block
    blk_nnodes = []
    blk_nnz = []
    cur_f, cur_n, cur_z = 0, 0, 0
    for i, c in enumerate(cnt):
        if cur_n == NBW or cur_z + c > 128:
            blk_first.append(cur_f)
            blk_nnodes.append(cur_n)
            blk_nnz.append(cur_z)
            cur_f, cur_n, cur_z = i, 0, 0
        cur_n += 1
        cur_z += int(c)
    blk_first.append(cur_f)
    blk_nnodes.append(cur_n)
    blk_nnz.append(cur_z)
    blk_first = np.array(blk_first)
    blk_nnodes = np.array(blk_nnodes)
    blk_nnz = np.array(blk_nnz)
    nblk = len(blk_first)

    # contiguous assignment of blocks to cores; same tile count everywhere
    bounds = [(c * nblk) // NCORES for c in range(NCORES + 1)]
    per_core = max(bounds[c + 1] - bounds[c] for c in range(NCORES))
    nt = -(-per_core // TPS) * TPS              # pad to whole strips
    core_of_blk = np.zeros(nblk, np.int64)
    t_of_blk = np.zeros(nblk, np.int64)
    for c in range(NCORES):
        core_of_blk[bounds[c]:bounds[c + 1]] = c
        t_of_blk[bounds[c]:bounds[c + 1]] = \
            np.arange(bounds[c + 1] - bounds[c])

    # per-nnz coordinates (blocks partition the sorted stream contiguously)
    blk_nnz_start = np.zeros(nblk, np.int64)
    blk_nnz_start[1:] = np.cumsum(blk_nnz)[:-1]
    blk_of_k = np.repeat(np.arange(nblk), blk_nnz)
    p_k = np.arange(NNZ) - blk_nnz_start[blk_of_k]
    nzrank = np.zeros(N_NODES, np.int64)
    nzrank[nz_nodes] = np.arange(len(nz_nodes))
    col_k = nzrank[rs] - blk_first[blk_of_k]
    core_k = core_of_blk[blk_of_k]
    t_k = t_of_blk[blk_of_k]

    x8 = x.astype(ml_dtypes.float8_e4m3)
    xg_cores = np.zeros((NCORES, 128, nt, C + NBW), dtype=np.uint8)
    xg_cores.view(ml_dtypes.float8_e4m3)[core_k, p_k, t_k, :C] = x8[cs]
    xg_cores[core_k, p_k, t_k, C + col_k] = FP8_ONE
    xg_cores = xg_cores.view(ml_dtypes.float8_e4m3)

    w1b = w1.astype(ml_dtypes.bfloat16)
    w2cb = np.zeros((C, 32), np.float32)
    w2cb[:, 0] = w2[:, 0]
    w2cb = w2cb.astype(ml_dtypes.bfloat16)

    prog_a = _get_prog(("A", nt), _build_prog_a, nt)
    in_maps = [{"xg": xg_cores[m], "w1": w1b,
                "w2c": w2cb} for m in range(NCORES)]
    res_a = _run(prog_a, in_maps, "A")

    # ---- host glue: assemble hw0, gather per-nonzero values ----
    parts = np.stack([res_a[m]["hw0"][0] for m in range(NCORES)])
    nzidx = np.arange(len(nz_nodes))
    blk_of_nz = np.repeat(np.arange(nblk), blk_nnodes)
    pos_in_blk = nzidx - blk_first[blk_of_nz]
    core_n = core_of_blk[blk_of_nz]
    col_n = t_of_blk[blk_of_nz] * NBW + pos_in_blk
    hw0 = np.zeros(N_NODES, np.float32)
    hw0[nz_nodes] = parts[core_n, col_n]
    zg = hw0[inc_rows.astype(np.int64)]
    za = zg[0::2]
    zb = zg[1::2]

    # ---- launch B: sigmoid + reduce ----
    FREE = -(-N_EDGES // (NCORES * 128))               # 196
    tot = NCORES * 128 * FREE
    zap = np.full(tot, -1.0e4, np.float32)
    zbp = np.full(tot, -1.0e4, np.float32)
    zap[:N_EDGES] = za
    zbp[:N_EDGES] = zb
    zab = np.concatenate(
        [zap.reshape(NCORES, 128, FREE), zbp.reshape(NCORES, 128, FREE)],
        axis=2).astype(ml_dtypes.bfloat16)

    prog_b = _get_prog(("B", FREE), _build_prog_b, FREE)
    in_maps_b = [{"zab": zab[m]} for m in range(NCORES)]
    res_b = _run(prog_b, in_maps_b, "B")

    total = float(sum(float(r["acc"].sum()) for r in res_b))
    return np.array(total / N_EDGES, dtype=np.float32)
